# revision 2
# baseline (speedup 1.0000x reference)
"""Trainium2 Bass kernel for nn_DispersiveLoss (B=2048, D=16*768=12288, 8 cores).

Strategy (circulant block decomposition, uniform SPMD, single launch):
  x (2048, 12288) -> 16 row-blocks of 128. Core c owns m-blocks {2c, 2c+1}
  and computes two Gram strips G[m, m..m+8 (mod 16)] (width 9 blocks = 1152)
  in fp8 DoubleRow (D on partitions, 48 two-chunk k-steps, PSUM fp32).
  Every unordered block pair lands exactly once (circular distance 1..7),
  diagonal blocks are masked to the upper triangle, distance-8 blocks are
  computed twice and weighted 0.5.

  Row norms sq are computed on the HOST (O(B*D) prep, like the transpose).
  The per-column correction v_j = (mean(sq) - sq_j)/2 is folded into PSUM
  by K=1 bf16 matmuls (one per PSUM tile); the per-row term rides the ACT
  bias in exact f32. PSUM thus holds P = g + v_j; ACT produces
  exp(2*SS*P + bias_i) and DVE produces sum(P), sum(P^2) per region with
  accum_out row-sums. Host reconstructs S1/S2 exactly in f64 (it knows the
  bf16 quantization residuals), so only fp8 Gram noise remains.

  The union columns stream in two phases (A: union cols 0:640 incl. both
  lhs blocks, kept resident; B: cols 640:1280, ring-buffered) so phase A's
  post-processing overlaps phase B's matmuls and the serial tail is only
  phase B's post (~3us).
"""

import os

import numpy as np
import ml_dtypes

import concourse.bass as bass
import concourse.mybir as mybir
import concourse.tile as tile
from concourse import bacc
from concourse.bass_utils import run_bass_kernel_spmd

NC_N = 8
B, D = 2048, 12288
BLK = 128
UNION = 1280
KCH = 96  # k-chunks of 128
TAU = 0.5
CC = float(2 * D)
SS = 1.0 / (D * TAU)
S2E = 2.0 * SS
F32 = mybir.dt.float32
BF16 = mybir.dt.bfloat16
FP8 = mybir.dt.float8e4
NP_FP8 = ml_dtypes.float8_e4m3
NP_BF16 = ml_dtypes.bfloat16
LN_HALF = float(np.log(0.5))
N_PAIRS = B * (B - 1) // 2

KERNEL_EXEC_NS = []  # filled when KERNEL_TRACE is set (test harness only)

_cache = {}


def _trace_enabled():
    return bool(os.environ.get("KERNEL_TRACE"))


def _build_kernel():
    nc = bacc.Bacc("TRN2", target_bir_lowering=False, debug=False, num_devices=NC_N)
    xA = nc.dram_tensor("xA", [BLK, KCH, 640], FP8, kind="ExternalInput")
    xB = nc.dram_tensor("xB", [BLK, KCH, 640], FP8, kind="ExternalInput")
    auxf = nc.dram_tensor("auxf", [BLK, 132], F32, kind="ExternalInput")
    auxb = nc.dram_tensor("auxb", [1, 1408], BF16, kind="ExternalInput")
    out_stats = nc.dram_tensor("out_stats", [BLK, 4], F32, kind="ExternalOutput")

    MULT = mybir.AluOpType.mult
    ADD = mybir.AluOpType.add
    EXP = mybir.ActivationFunctionType.Exp
    DR = mybir.MatmulPerfMode.DoubleRow

    # A-phase DMA batches (chunks): small first so PE starts early
    ABATCH = [(0, 2), (2, 14), (14, 26), (26, 38), (38, 50), (50, 62),
              (62, 74), (74, 86), (86, 96)]
    BBATCH = [(k, k + 12) for k in range(0, 96, 12)]

    # acc columns: E 0..8, S1s0 9..13, S1s1 14..17, S2 18..26
    with tile.TileContext(nc) as tc:
        with (
            tc.tile_pool(name="g", bufs=1) as g,
            tc.tile_pool(name="bp", bufs=3) as bp,
            tc.tile_pool(name="sp", bufs=4) as sp,
            tc.tile_pool(name="ps", bufs=1, space="PSUM") as psp,
        ):
            tA = g.tile([BLK, KCH, 640], FP8)
            auxf_t = g.tile([BLK, 132], F32)
            auxb_t = g.tile([1, 1408], BF16)
            acc = g.tile([BLK, 27], F32)
            nc.sync.dma_start(auxf_t[:], auxf[:])
            nc.sync.dma_start(auxb_t[:], auxb[:])
            tri_t = auxf_t[:, 0:128]
            bias0 = auxf_t[:, 128:129]
            bias1 = auxf_t[:, 129:130]
            biash0 = auxf_t[:, 130:131]
            biash1 = auxf_t[:, 131:132]
            vq = auxb_t  # cols 0:1280 = v_q (union order), 1280:1408 = ones
            ones = auxb_t[:, 1280:1408]

            # preload Exp activation table early (off critical path)
            zcol = g.tile([BLK, 1], F32)
            nc.gpsimd.memset(zcol[:], 0.0)
            zscr = g.tile([BLK, 1], F32)
            nc.scalar.activation(zscr[:], zcol[:], EXP)

            # PSUM tiles
            tA0 = psp.tile([BLK, 512], F32, tag="tA0")  # s0 x union[0:512)
            tA1 = psp.tile([BLK, 128], F32, tag="tA1")  # s0 x union[512:640)
            tA2 = psp.tile([BLK, 512], F32, tag="tA2")  # s1 x union[128:640)
            tB0 = psp.tile([BLK, 512], F32, tag="tB0")  # s0 x union[640:1152)
            tB1 = psp.tile([BLK, 512], F32, tag="tB1")  # s1 x union[640:1152)
            tB2 = psp.tile([BLK, 128], F32, tag="tB2")  # s1 x union[1152:1280)

            # ---- phase A: stream + matmul (tA stays resident) ----
            for (k0, k1) in ABATCH:
                nc.sync.dma_start(tA[:, k0:k1, :], xA[:, k0:k1, :])
                for k in range(k0, k1, 2):
                    lhs0 = tA[:, k : k + 2, 0:128]
                    lhs1 = tA[:, k : k + 2, 128:256]
                    st = (k == 0)
                    nc.tensor.matmul(tA0[:], lhs0, tA[:, k : k + 2, 0:512],
                                     start=st, stop=False, perf_mode=DR)
                    nc.tensor.matmul(tA1[:], lhs0, tA[:, k : k + 2, 512:640],
                                     start=st, stop=False, perf_mode=DR)
                    nc.tensor.matmul(tA2[:], lhs1, tA[:, k : k + 2, 128:640],
                                     start=st, stop=False, perf_mode=DR)
            # fold v (bf16 K=1): P += ones^T @ vq
            nc.tensor.matmul(tA0[:], ones, vq[:, 0:512], start=False, stop=True)
            nc.tensor.matmul(tA1[:], ones, vq[:, 512:640], start=False, stop=True)
            nc.tensor.matmul(tA2[:], ones, vq[:, 128:640], start=False, stop=True)

            # ---- post A (overlaps phase B matmuls) ----
            def post_full(pm, w, bias, e_i, s1_i, s2_i, wt=1.0):
                scr = sp.tile([BLK, w], F32, tag="scr")
                nc.scalar.activation(scr[:], pm, EXP, bias=bias, scale=S2E,
                                     accum_out=acc[:, e_i : e_i + 1])
                cp = sp.tile([BLK, w], F32, tag="cp")
                nc.vector.tensor_scalar(
                    out=cp[:], in0=pm, scalar1=wt, scalar2=0.0, op0=MULT, op1=ADD,
                    accum_out=acc[:, s1_i : s1_i + 1])
                sq2 = sp.tile([BLK, w], F32, tag="sq2")
                nc.vector.scalar_tensor_tensor(
                    out=sq2[:], in0=cp[:], scalar=1.0, in1=pm, op0=MULT, op1=MULT,
                    accum_out=acc[:, s2_i : s2_i + 1])

            def post_diag(pd, bias, e_i, s1_i, s2_i):
                et = sp.tile([BLK, BLK], F32, tag="et")
                nc.scalar.activation(et[:], pd, EXP, bias=bias, scale=S2E)
                me = sp.tile([BLK, BLK], F32, tag="me")
                nc.vector.scalar_tensor_tensor(
                    out=me[:], in0=et[:], scalar=1.0, in1=tri_t, op0=MULT, op1=MULT,
                    accum_out=acc[:, e_i : e_i + 1])
                mu = sp.tile([BLK, BLK], F32, tag="mu")
                nc.vector.scalar_tensor_tensor(
                    out=mu[:], in0=pd, scalar=1.0, in1=tri_t, op0=MULT, op1=MULT,
                    accum_out=acc[:, s1_i : s1_i + 1])
                ms2 = sp.tile([BLK, BLK], F32, tag="ms2")
                nc.vector.scalar_tensor_tensor(
                    out=ms2[:], in0=mu[:], scalar=1.0, in1=pd, op0=MULT, op1=MULT,
                    accum_out=acc[:, s2_i : s2_i + 1])

            post_diag(tA0[:, 0:128], bias0, 1, 10, 19)
            post_full(tA0[:, 128:512], 384, bias0, 0, 9, 18)
            post_full(tA1[:], 128, bias0, 2, 11, 20)
            post_diag(tA2[:, 0:128], bias1, 4, 15, 22)
            post_full(tA2[:, 128:512], 384, bias1, 3, 14, 21)

            # ---- phase B: stream (ring) + matmul; lhs from resident tA ----
            first = True
            for (k0, k1) in BBATCH:
                bt = bp.tile([BLK, 12, 640], FP8, tag="bt")
                nc.sync.dma_start(bt[:], xB[:, k0:k1, :])
                for j in range(0, 12, 2):
                    k = k0 + j
                    lhs0 = tA[:, k : k + 2, 0:128]
                    lhs1 = tA[:, k : k + 2, 128:256]
                    nc.tensor.matmul(tB0[:], lhs0, bt[:, j : j + 2, 0:512],
                                     start=first, stop=False, perf_mode=DR)
                    nc.tensor.matmul(tB1[:], lhs1, bt[:, j : j + 2, 0:512],
                                     start=first, stop=False, perf_mode=DR)
                    nc.tensor.matmul(tB2[:], lhs1, bt[:, j : j + 2, 512:640],
                                     start=first, stop=False, perf_mode=DR)
                    first = False
            nc.tensor.matmul(tB0[:], ones, vq[:, 640:1152], start=False, stop=True)
            nc.tensor.matmul(tB1[:], ones, vq[:, 640:1152], start=False, stop=True)
            nc.tensor.matmul(tB2[:], ones, vq[:, 1152:1280], start=False, stop=True)

            # ---- post B (serial tail) ----
            post_full(tB0[:, 0:384], 384, bias0, 5, 12, 23)
            post_full(tB0[:, 384:512], 128, biash0, 6, 13, 24, wt=0.5)
            post_full(tB1[:], 512, bias1, 7, 16, 25)
            post_full(tB2[:], 128, biash1, 8, 17, 26, wt=0.5)

            outt = g.tile([BLK, 4], F32)
            nc.vector.tensor_reduce(outt[:, 0:1], acc[:, 0:9],
                                    mybir.AxisListType.X, ADD)
            nc.vector.tensor_reduce(outt[:, 1:2], acc[:, 9:14],
                                    mybir.AxisListType.X, ADD)
            nc.vector.tensor_reduce(outt[:, 2:3], acc[:, 14:18],
                                    mybir.AxisListType.X, ADD)
            nc.vector.tensor_reduce(outt[:, 3:4], acc[:, 18:27],
                                    mybir.AxisListType.X, ADD)
            nc.sync.dma_start(out_stats[:], outt[:])
    nc.compile()
    return nc


def _get(name, builder):
    if name not in _cache:
        _cache[name] = builder()
    return _cache[name]


def _run(nc, in_maps, tag):
    if _trace_enabled():
        try:
            import profhook

            profhook.install()
        except Exception:
            pass
        import tempfile

        res = run_bass_kernel_spmd(
            nc, in_maps, list(range(NC_N)), trace=True,
            tmpdir=tempfile.mkdtemp(prefix=f"ktrace_{tag}_"),
        )
        KERNEL_EXEC_NS.append((tag, res.exec_time_ns))
        return res.results
    return run_bass_kernel_spmd(nc, in_maps, list(range(NC_N))).results


def kernel(features):
    x = np.asarray(features).reshape(B, D)
    xq8 = x.astype(NP_FP8)

    # host-side prep (f64 exact)
    xqf = xq8.astype(np.float32)
    sq = (xqf.astype(np.float64) ** 2).sum(1)
    Mbar = sq.mean()
    a = sq + Mbar - CC
    vprime = (Mbar - sq) / 2.0
    vq16 = vprime.astype(NP_BF16)
    vqf = vq16.astype(np.float64)
    delta = vprime - vqf

    xT_full = np.ascontiguousarray(xq8.T)  # (D, B)
    tri = np.triu(np.ones((BLK, BLK), np.float32), k=1)
    in_maps = []
    col_list = []
    for c in range(NC_N):
        cols = (256 * c + np.arange(UNION)) % B
        col_list.append(cols)
        xu = xT_full[:, cols].reshape(KCH, BLK, UNION).transpose(1, 0, 2)
        rows0 = np.arange(256 * c, 256 * c + 128)
        rows1 = rows0 + 128
        auxf = np.empty((BLK, 132), np.float32)
        auxf[:, 0:128] = tri
        auxf[:, 128] = (-SS * a[rows0]).astype(np.float32)
        auxf[:, 129] = (-SS * a[rows1]).astype(np.float32)
        auxf[:, 130] = auxf[:, 128] + LN_HALF
        auxf[:, 131] = auxf[:, 129] + LN_HALF
        auxb = np.empty((1, 1408), NP_BF16)
        auxb[0, 0:1280] = vq16[cols]
        auxb[0, 1280:1408] = NP_BF16(1.0)
        in_maps.append({
            "xA": np.ascontiguousarray(xu[:, :, 0:640]),
            "xB": np.ascontiguousarray(xu[:, :, 640:1280]),
            "auxf": auxf,
            "auxb": auxb,
        })

    nc = _get("main", _build_kernel)
    res = _run(nc, in_maps, "main")

    # ---- host combine (f64) ----
    E_tot = 0.0
    S1_tot = 0.0
    S2_tot = 0.0
    m_i = 1087.0 - np.arange(128)
    n_mult = np.full(1152, 128.0)
    n_mult[0:128] = np.arange(128)
    n_mult[1024:1152] = 64.0
    for c in range(NC_N):
        o = res[c]["out_stats"].astype(np.float64)
        cols = col_list[c]
        rows0 = np.arange(256 * c, 256 * c + 128)
        rows1 = rows0 + 128
        R0 = o[:, 1]
        R1 = o[:, 2]
        E_tot += o[:, 0].sum()
        P2 = o[:, 3].sum()
        d_u0 = delta[cols[0:1152]]
        d_u1 = delta[cols[128:1280]]
        S1_c = (-2.0 * (R0.sum() + R1.sum())
                + (m_i * (a[rows0] + a[rows1])).sum()
                - 2.0 * ((n_mult * d_u0).sum() + (n_mult * d_u1).sum()))
        mid = (a[rows0] * R0).sum() + (a[rows1] * R1).sum()
        cum0 = np.cumsum(d_u0[0:128][::-1])[::-1]
        rowd0 = (np.concatenate([cum0[1:], [0.0]])
                 + d_u0[128:1024].sum() + 0.5 * d_u0[1024:1152].sum())
        cum1 = np.cumsum(d_u1[0:128][::-1])[::-1]
        rowd1 = (np.concatenate([cum1[1:], [0.0]])
                 + d_u1[128:1024].sum() + 0.5 * d_u1[1024:1152].sum())
        C2 = ((m_i * (a[rows0] ** 2 + a[rows1] ** 2)).sum()
              - 4.0 * ((a[rows0] * rowd0).sum() + (a[rows1] * rowd1).sum())
              + 4.0 * ((n_mult * d_u0 ** 2).sum() + (n_mult * d_u1 ** 2).sum()))
        S1_tot += S1_c
        S2_tot += 4.0 * P2 - 4.0 * mid + C2

    N = float(N_PAIRS)
    mean_u = S1_tot / N
    mean = (mean_u + CC) / D
    var_u = (S2_tot - N * mean_u * mean_u) / (N - 1.0)
    std = np.sqrt(var_u) / D
    loss = CC * SS - np.log(E_tot) + np.log(N)
    feat_norm = np.sqrt((x.astype(np.float64) ** 2).sum(1)).mean()

    return (
        np.float32(loss),
        np.float32(feat_norm),
        np.float32(mean),
        np.float32(std),
    )


if __name__ == "__main__":
    f = np.random.default_rng(0).standard_normal((B, 16, 768), dtype=np.float32)
    print(kernel(features=f))
